# revision 21
# baseline (speedup 1.0000x reference)
"""Trainium2 Bass kernel for the blockwise spiking network (nn_Blocks_86096914416140).

Sharding: data-parallel over batch B=32 across 8 NeuronCores (4 batches/core),
all parameters replicated, zero collectives.

Per-core device algorithm (fp32 throughout; channel-on-partition layout:
partition = c % 128, tiles [128, (c_hi=4, b=4, t=32)]):

  block n:   xr   = x_blk + W @ spikes_prev + beta*v_init@t0         [TensorE]
             q    = cnt_prev + (1 - spiked_prev)                      [GpSimd]
             cur  = min(q, 1) * xr          (refractory gate)         [VectorE]
             mem  = scan: s = beta*s + cur   (seg-reset via pattern)  [VectorE]
             fs   = (mem - 1) > thr          thr = a*(b*p^{t+1})      [VectorE]
             cnt  = seg-cumsum(fs)                                    [VectorE]
             spk  = (cnt == 1) * fs          (bf16, exact 0/1)        [VectorE]
             pdec = scan: s = p*s + spk  -> last col = p^{31-t0}      [VectorE]
             a'   = p^32 * a + (1/p) * pdec_last                      [GpSimd]
             thr' = a' * (b*p^{t+1})                                  [GpSimd]
             spikes regrouped to 8-block chunks for wide DMA          [ScalarE]

The recurrent weight streams through the PE as an exact 3-way bf16 split
(w1+w2+w3 == W to ~2^-27; spike operand is exactly representable in bf16, so
every product is exact and PSUM accumulates in fp32) — 4x the fp32 matmul
streaming rate with fp32-equivalent rounding. The fp32 x tile is added via an
identity matmul into the same PSUM accumulation group; v_init (pre-scaled by
beta on GpSimd) is injected into the t=0 columns the same way.

All tables are precomputed on the host in fp32 to match the reference's
rounding. Validated bit-exact (zero spike flips) against the fp32 reference
both in CoreSim and on the 8 hardware NeuronCores.
"""

import numpy as np

B, C, T_LEN, T = 32, 512, 1024, 32
NB = T_LEN // T          # 32 blocks
NCORES = 8
BPC = B // NCORES        # 4 batches per core
CH = C // 128            # 4 channel tiles
FREE = CH * BPC * T      # 512 free elements per full tile
BP = BPC // 2            # batches per pipeline
FREE2 = CH * BP * T      # 256 free elements per pipeline tile
GRP = 4                  # blocks per x-load group
NGRP = NB // GRP         # 4 DMA groups
GT = GRP * T             # 256 t-cols per group

_compiled = None


def _build_program():
    import concourse.bass as bass
    import concourse.bacc as bacc
    import concourse.tile as tile
    from concourse import mybir
    from concourse._compat import with_exitstack
    from contextlib import ExitStack

    f32 = mybir.dt.float32
    Alu = mybir.AluOpType

    bf16 = mybir.dt.bfloat16

    # fp32 const blob layout: [128, CBLOB] with slices at fixed offsets
    OF_BETAT = 0
    OF_P32 = OF_BETAT + CH
    OF_BSEG = OF_P32 + CH
    OF_PSEG = OF_BSEG + FREE2
    OF_SEG01 = OF_PSEG + FREE2
    OF_BP1 = OF_SEG01 + FREE2
    OF_ID = OF_BP1 + FREE2
    OF_BDIAG = OF_ID + 128
    CBLOB = OF_BDIAG + CH * 128

    nc = bacc.Bacc()
    # x / out pre-tiled on host: [NGRP, CH, 128, BPC*GRP*T] (contiguous DMA)
    x_d = nc.declare_dram_parameter("x_sh", [NGRP, CH, 128, BPC * GT], f32,
                                    isOutput=False)
    wt_d = nc.declare_dram_parameter("wt", [128, 48, 128], bf16, isOutput=False)
    cblob_d = nc.declare_dram_parameter("cblob", [128, CBLOB], f32,
                                        isOutput=False)
    out_d = nc.declare_dram_parameter("out", [NGRP, CH, 128, BPC * GT], f32,
                                      isOutput=True)

    def flat(ap4):
        return ap4.rearrange("p c b t -> p (c b t)")

    @with_exitstack
    def kern(ctx: ExitStack, tc: tile.TileContext):
        consts = ctx.enter_context(tc.tile_pool(name="consts", bufs=1))
        xpool = ctx.enter_context(tc.tile_pool(name="xpool", bufs=2))
        work = ctx.enter_context(tc.tile_pool(name="work", bufs=2))
        spkp = ctx.enter_context(tc.tile_pool(name="spkp", bufs=3))
        small = ctx.enter_context(tc.tile_pool(name="small", bufs=3))
        psum = ctx.enter_context(tc.tile_pool(name="psum", bufs=2, space="PSUM"))

        dma = nc.sync

        cblob_t = consts.tile([128, CBLOB], f32, tag="cblob")
        cblob_src = cblob_d  # DMA deferred until after the first x-group loads
        wt_t = consts.tile([128, 48, 128], bf16, tag="wt")
        wt_src = wt_d  # DMA deferred until after the first x-group loads
        betat_t = cblob_t[:, OF_BETAT:OF_BETAT + CH].rearrange(
            "p (c u v) -> p c u v", u=1, v=1)
        p32_t = cblob_t[:, OF_P32:OF_P32 + CH].rearrange(
            "p (c u v) -> p c u v", u=1, v=1)
        betaseg_t = cblob_t[:, OF_BSEG:OF_BSEG + FREE2]
        pseg_t = cblob_t[:, OF_PSEG:OF_PSEG + FREE2]
        seg01_t = cblob_t[:, OF_SEG01:OF_SEG01 + FREE2]
        bp1_t = cblob_t[:, OF_BP1:OF_BP1 + FREE2].rearrange(
            "p (c b t) -> p c b t", c=CH, b=BP)
        id_t = cblob_t[:, OF_ID:OF_ID + 128]
        bdiag_t = cblob_t[:, OF_BDIAG:OF_BDIAG + CH * 128].rearrange(
            "p (c m) -> p c m", c=CH)

        thr0_t = consts.tile([128, CH, BP, T], f32, tag="thr0")
        nc.vector.memset(thr0_t[:], 0.0)

        import os
        for _rep in range(int(os.environ.get("BENCH_REPEAT", "1"))):
            _run_once(nc, tc, locals())

    def _run_once(nc, tc, env):
        consts = env["consts"]; xpool = env["xpool"]; work = env["work"]
        spkp = env["spkp"]; small = env["small"]; psum = env["psum"]
        dma = env["dma"]
        wt_t = env["wt_t"]; wt_src = env["wt_src"]
        cblob_t = env["cblob_t"]; cblob_src = env["cblob_src"]
        OF_BP1 = env["OF_BP1"]
        betat_t = env["betat_t"]; id_t = env["id_t"]
        bdiag_t = env["bdiag_t"]
        betaseg_t = env["betaseg_t"]; pseg_t = env["pseg_t"]
        seg01_t = env["seg01_t"]; bp1_t = env["bp1_t"]
        p32_t = env["p32_t"]; thr0_t = env["thr0_t"]

        NP = 2            # batch-pair pipelines per core

        # per-pipeline persistent state (block n-1 -> n)
        a_t = [None] * NP
        thr_t = [None] * NP
        vinit_t = [None] * NP
        q_t = [None] * NP
        spk_prev = [None] * NP

        x_g = None
        sgrp = None
        for n in range(NB):
            gi, go = divmod(n, GRP)
            if go == 0:
                x_g = xpool.tile([128, CH, BPC, GRP * T], f32, tag="xg")
                if n == 0:
                    # wt first: big transfer, and this ordering means block 1
                    # never stalls on it while block 0's chain runs
                    dma.dma_start(out=wt_t[:], in_=wt_src[:])
                for ci in range(CH):
                    dma.dma_start(
                        out=x_g[:, ci],
                        in_=x_d[gi, ci].rearrange("p (b t) -> p b t", b=BPC))
                sgrp = xpool.tile([128, CH, BPC, GRP, T], f32, tag="sgrp")
                if n == 0:
                    dma.dma_start(out=cblob_t[:, :OF_BP1],
                                  in_=cblob_src[:, :OF_BP1])
                    dma.dma_start(out=cblob_t[:, OF_BP1:],
                                  in_=cblob_src[:, OF_BP1:])

            # per-pipeline tiles (separate tiles avoid cross-pipeline deps)
            cur_t = [work.tile([128, CH, BP, T], f32, tag=f"cur{h}",
                               name=f"cur{h}") for h in range(NP)]
            mem_t = [work.tile([128, CH, BP, T], f32, tag=f"mem{h}",
                               name=f"mem{h}") for h in range(NP)]
            fs_t = [work.tile([128, CH, BP, T], f32, tag=f"fs{h}",
                              name=f"fs{h}") for h in range(NP)]
            cnt_t = [work.tile([128, CH, BP, T], f32, tag=f"cnt{h}",
                               name=f"cnt{h}") for h in range(NP)]
            spk_t = [spkp.tile([128, CH, BP, T], bf16, tag=f"spk{h}",
                               name=f"spk{h}") for h in range(NP)]

            def xsl(h):  # x slice for pipeline h, this block
                return x_g[:, :, h * BP:(h + 1) * BP, go * T:(go + 1) * T]

            def mm_phase(h):
                # xr = x + W @ spikes_prev + beta*v_init@t0 for pipeline h
                xr = psum.tile([128, CH, BP, T], f32, tag=f"xr{h}")
                nc.tensor.matmul(out=xr[:], lhsT=id_t[:], rhs=xsl(h),
                                 start=True, stop=False)
                # same per-element accumulation order as the validated
                # baseline: (v, cl, cj in (0,1)), then (v, cl, cj in (2,3))
                for cjs in ((0, 1), (2, 3)):
                    for v in range(3):
                        for cl in range(CH):
                            for cj in cjs:
                                nc.tensor.matmul(
                                    out=xr[:, cl],
                                    lhsT=wt_t[:, v * 16 + cj * CH + cl],
                                    rhs=spk_prev[h][:, cj],
                                    start=False, stop=False)
                for ci in range(CH):
                    nc.tensor.matmul(
                        out=xr[:, ci, :, 0:1], lhsT=bdiag_t[:, ci],
                        rhs=vinit_t[h][:, ci].rearrange("p b u -> p (b u)"),
                        start=False, stop=(ci == CH - 1))
                return xr

            def chain(h, xr):
                if n == 0:
                    nc.vector.tensor_copy(out=cur_t[h][:], in_=xsl(h))
                else:
                    # refractory gate: cur = min(q, 1) * xr
                    nc.vector.scalar_tensor_tensor(
                        out=cur_t[h][:], in0=q_t[h][:], scalar=1.0,
                        in1=xr[:], op0=Alu.min, op1=Alu.mult)
                nc.vector.tensor_tensor_scan(
                    out=flat(mem_t[h]), data0=betaseg_t[:],
                    data1=flat(cur_t[h]),
                    initial=0.0, op0=Alu.mult, op1=Alu.add)
                nc.vector.scalar_tensor_tensor(
                    out=fs_t[h][:], in0=mem_t[h][:], scalar=1.0,
                    in1=(thr_t[h][:] if n > 0 else thr0_t[:]),
                    op0=Alu.subtract, op1=Alu.is_gt)
                nc.vector.tensor_tensor_scan(
                    out=flat(cnt_t[h]), data0=seg01_t[:],
                    data1=flat(fs_t[h]),
                    initial=0.0, op0=Alu.mult, op1=Alu.add)
                for cjh in range(2):
                    nc.vector.scalar_tensor_tensor(
                        out=spk_t[h][:, cjh * 2:cjh * 2 + 2],
                        in0=cnt_t[h][:, cjh * 2:cjh * 2 + 2], scalar=1.0,
                        in1=fs_t[h][:, cjh * 2:cjh * 2 + 2],
                        op0=Alu.is_equal, op1=Alu.mult)

            def pool_phase(h, pdec_t):
                # ns = (cnt[T-1] == 0);  gate q = ns + cnt
                ns_new = small.tile([128, CH, BP, 1], f32, tag=f"ns{h}")
                nc.gpsimd.tensor_single_scalar(
                    out=ns_new[:], in_=cnt_t[h][:, :, :, T - 1:T],
                    scalar=0.0, op=Alu.is_equal)
                q_new = work.tile([128, CH, BP, T], f32, tag=f"gate{h}")
                nc.gpsimd.tensor_tensor(
                    out=q_new[:], in0=ns_new.broadcast_to([128, CH, BP, T]),
                    in1=cnt_t[h][:], op=Alu.add)

                # v_init = (cnt[T-1]==0) * mem[T-1]; beta fold happens in
                # the beta-diag v_init matmul
                vinit_new = small.tile([128, CH, BP, 1], f32, tag=f"vinit{h}")
                nc.gpsimd.tensor_tensor(
                    out=vinit_new[:], in0=ns_new[:],
                    in1=mem_t[h][:, :, :, T - 1:T], op=Alu.mult)

                # ahat := p * a  =>  ahat' = p^32 * ahat + pdec_last,
                # thr = ahat * (b * p^t)   (1/p folded into the bp1 table)
                a_new = small.tile([128, CH, BP, 1], f32, tag=f"a{h}")
                pl = pdec_t[:, :, :, T - 1:T]
                if n == 0:
                    nc.gpsimd.tensor_copy(out=a_new[:], in_=pl)
                else:
                    v_t = small.tile([128, CH, BP, 1], f32, tag=f"v{h}")
                    nc.gpsimd.tensor_tensor(
                        out=v_t[:], in0=a_t[h][:],
                        in1=p32_t.broadcast_to([128, CH, BP, 1]), op=Alu.mult)
                    nc.gpsimd.tensor_tensor(
                        out=a_new[:], in0=pl, in1=v_t[:], op=Alu.add)

                thr_new = work.tile([128, CH, BP, T], f32, tag=f"thr{h}")
                nc.gpsimd.tensor_tensor(
                    out=thr_new[:],
                    in0=a_new.broadcast_to([128, CH, BP, T]),
                    in1=bp1_t[:], op=Alu.mult)

                a_t[h], thr_t[h], vinit_t[h], q_t[h] = (
                    a_new, thr_new, vinit_new, q_new)

            for h in range(NP):
                xr = mm_phase(h) if n > 0 else None
                chain(h, xr)
                nc.scalar.copy(out=sgrp[:, :, h * BP:(h + 1) * BP, go],
                               in_=spk_t[h][:])
                spk_prev[h] = spk_t[h]
            if n < NB - 1:
                # adaptation decay scans (DVE; fill the post-chain window)
                pdec_t = [work.tile([128, CH, BP, T], f32, tag=f"pdec{h}",
                                    name=f"pdec{h}") for h in range(NP)]
                for h in range(NP):
                    nc.vector.tensor_tensor_scan(
                        out=flat(pdec_t[h]), data0=pseg_t[:],
                        data1=flat(spk_t[h]),
                        initial=0.0, op0=Alu.mult, op1=Alu.add)
                for h in range(NP):
                    pool_phase(h, pdec_t[h])

            if go == GRP - 1:
                for ci in range(CH):
                    dma.dma_start(
                        out=out_d[gi, ci].rearrange("p (b t) -> p b t", b=BPC),
                        in_=sgrp[:, ci])

    with tile.TileContext(nc) as tc:
        kern(tc)
    nc.compile()
    return nc


def _host_tables(beta_raw, rec_weight, p_raw, b_raw):
    f = np.float32
    W = rec_weight.astype(f)
    beta = np.clip(beta_raw.astype(f), f(0.001), f(0.999))
    p = np.clip(np.abs(p_raw.astype(f)), f(0.0), f(0.999))
    bb = np.clip(np.abs(b_raw.astype(f)), f(0.001), f(1.0))
    p_pow = (p[:, None] ** np.arange(1, T + 1, dtype=f)).astype(f)   # (C,T)
    # thr = ahat * b * p^t with ahat = p*a  (1/p folded into the table)
    BP1 = (bb[:, None] * (p[:, None] ** np.arange(0, T, dtype=f))).astype(f)
    p32 = np.ascontiguousarray(p_pow[:, -1])

    def per_ct(vals_ct):  # (C,T) -> (128, CH*BP*T), replicated over b-pair
        v = vals_ct.reshape(CH, 128, T)
        out = np.zeros((128, CH, BP, T), f)
        out[:] = v.transpose(1, 0, 2)[:, :, None, :]
        return np.ascontiguousarray(out.reshape(128, FREE2))

    t0mask = np.ones((1, T), f)
    t0mask[0, 0] = 0.0
    betaseg = per_ct((beta[:, None] * t0mask).astype(f))
    pseg = per_ct((p[:, None] * t0mask).astype(f))
    seg01 = per_ct(np.broadcast_to(t0mask, (C, T)).astype(f))
    bp1 = per_ct(BP1)

    def per_c(vals_c):  # (C,) -> (128, CH)
        return np.ascontiguousarray(vals_c.reshape(CH, 128).T)

    # wt[cj_hi*CH + ci_hi][cj_lo, ci_lo] = W[ci_hi*128+ci_lo, cj_hi*128+cj_lo]
    import ml_dtypes
    W4 = W.reshape(CH, 128, CH, 128)
    wt16 = np.ascontiguousarray(
        W4.transpose(2, 0, 3, 1).reshape(16, 128, 128))
    # exact 3-way bf16 decomposition: w1+w2+w3 == W to ~2^-27 relative
    w1 = wt16.astype(ml_dtypes.bfloat16)
    r1 = wt16 - w1.astype(f)
    w2 = r1.astype(ml_dtypes.bfloat16)
    r2 = r1 - w2.astype(f)
    w3 = r2.astype(ml_dtypes.bfloat16)
    wt = np.ascontiguousarray(np.concatenate([w1, w2, w3], axis=0))
    ident = np.eye(128, dtype=f)
    # bdiag[p, ci, m] = beta[ci*128+p] if m == p else 0
    bdiag = np.zeros((128, CH, 128), f)
    for ci in range(CH):
        bdiag[np.arange(128), ci, np.arange(128)] = beta[ci * 128:(ci + 1) * 128]
    cblob = np.concatenate([
        per_c(beta), per_c(p32), betaseg, pseg, seg01, bp1, ident,
        bdiag.reshape(128, CH * 128)], axis=1)
    return dict(wt=np.ascontiguousarray(wt.transpose(1, 0, 2)),
                cblob=np.ascontiguousarray(cblob))


def kernel(x, beta_raw, rec_weight, p_raw, b_raw):
    global _compiled
    from concourse.bass_utils import run_bass_kernel_spmd

    if _compiled is None:
        _compiled = _build_program()
    nc = _compiled

    tables = _host_tables(np.asarray(beta_raw), np.asarray(rec_weight),
                          np.asarray(p_raw), np.asarray(b_raw))
    x = np.asarray(x).astype(np.float32)
    in_maps = []
    for k in range(NCORES):
        xc = x[k * BPC:(k + 1) * BPC]                     # (BPC, C, T_LEN)
        xt = xc.reshape(BPC, CH, 128, NGRP, GRP * T).transpose(3, 1, 2, 0, 4)
        m = {"x_sh": np.ascontiguousarray(
            xt.reshape(NGRP, CH, 128, BPC * GRP * T))}
        m.update(tables)
        in_maps.append(m)
    res = run_bass_kernel_spmd(nc, in_maps, list(range(NCORES)))
    outs = []
    for k in range(NCORES):
        o = res.results[k]["out"].reshape(NGRP, CH, 128, BPC, GRP * T)
        o = o.transpose(3, 1, 2, 0, 4).reshape(BPC, C, T_LEN)
        outs.append(o)
    return np.ascontiguousarray(np.concatenate(outs, axis=0)).astype(np.float32)

